# revision 1
# baseline (speedup 1.0000x reference)
"""Trainium2 Bass kernel for nn_LGnet (LSTM + memory attention recurrence).

Sharding: data-parallel over batch, B=256 -> 32 rows per core across 8 cores.
All on-chip state is kept transposed ([feature partitions, batch free]) so the
recurrence never needs a transpose. The z/zp gating streams (input-only) and
their contribution to the attention query `ls` are precomputed in T-chunks
before the sequential loop; the loop itself runs 100 steps of:
  ls = ls_z[t] + h @ WQ3F.T        (4 fp32 matmuls)
  logits = memory @ ls             (4 fp32 matmuls)
  e = exp(logits); s = sum(e); gd = (e @ memory) / s   (bf16 matmuls)
  gates = Wcat.T @ [gd; h]         (80 bf16 matmuls, weights stationary)
  LSTM pointwise via tanh (sigmoid = 0.5*tanh(0.5x)+0.5, ACT exp-table set)
"""
import os
import numpy as np
from contextlib import ExitStack

B, T, F, H, O, M = 256, 100, 128, 512, 128, 512
T = int(os.environ.get("LG_T", str(T)))   # debug override; harness uses 100
NC = 8
BB = B // NC          # 32 batch rows per core
TB = T * BB           # 3200 columns in (t, b) packing
NTCH = 4              # precompute T-chunks
TCH = T // NTCH       # 25 steps per chunk
CCH = TCH * BB        # 800 cols per chunk

_built = None


def _build():
    import concourse.bass as bass
    import concourse.tile as tile
    from concourse import bacc, mybir

    f32 = mybir.dt.float32
    bf16 = mybir.dt.bfloat16
    AF = mybir.ActivationFunctionType
    ALU = mybir.AluOpType
    nc = bacc.Bacc("TRN2", target_bir_lowering=False, debug=False, num_devices=NC)
    # ---- DRAM tensors (per-core data fed via in_maps) ----
    dt_in = {}
    for name in ["x", "xl", "mask", "delta", "xlb", "dltb", "xmb"]:
        dt_in[name] = nc.dram_tensor(name, [F, TB], f32, kind="ExternalInput").ap()
    wg_d = nc.dram_tensor("wg", [128, 80 * 128], bf16, kind="ExternalInput").ap()
    # bf16 declared below after dtype aliases
    wq3f_d = nc.dram_tensor("wq3f", [128, 512], f32, kind="ExternalInput").ap()
    memt_d = nc.dram_tensor("memt", [128, 512], f32, kind="ExternalInput").ap()
    membf_d = nc.dram_tensor("membf", [128, 512], bf16, kind="ExternalInput").ap()
    wfct_d = nc.dram_tensor("wfct", [128, 512], f32, kind="ExternalInput").ap()
    wqz_d = nc.dram_tensor("wqz", [128, 128], f32, kind="ExternalInput").ap()
    wqzp_d = nc.dram_tensor("wqzp", [128, 128], f32, kind="ExternalInput").ap()
    biast_d = nc.dram_tensor("biast", [128, 16], f32, kind="ExternalInput").ap()
    scal_d = nc.dram_tensor("scal", [128, 8], f32, kind="ExternalInput").ap()
    # scal cols: 0 dgz, 1 bgz, 2 dgzp, 3 bgzp, 4 b_q_eff, 5 b_fc
    o_d = nc.dram_tensor("o", [O, BB], f32, kind="ExternalOutput").ap()
    dbg = os.environ.get("LG_DEBUG") == "1"
    if dbg:
        dbg_d = {nm: nc.dram_tensor(f"dbg_{nm}", shp, f32, kind="ExternalOutput").ap()
                 for nm, shp in [("lsf", [128, BB]), ("eT", [128, 128]),
                                 ("ssb", [128, BB]), ("gdn", [128, BB]),
                                 ("Y", [128, 512]), ("h", [128, 128]),
                                 ("lsz", [128, BB]), ("z", [128, BB]), ("zp", [128, BB]),
                                 ("G", [128, 512]), ("hbin", [128, 128])]}

    with tile.TileContext(nc) as tc, ExitStack() as ctx:
        wpool = ctx.enter_context(tc.tile_pool(name="wpool", bufs=1))
        inp = ctx.enter_context(tc.tile_pool(name="inp", bufs=2))
        pre = ctx.enter_context(tc.tile_pool(name="pre", bufs=2))
        lszp = ctx.enter_context(tc.tile_pool(name="lszp", bufs=1))
        stp = ctx.enter_context(tc.tile_pool(name="stp", bufs=2))
        state = ctx.enter_context(tc.tile_pool(name="state", bufs=2))
        pers = ctx.enter_context(tc.tile_pool(name="pers", bufs=1))
        attn_ps = ctx.enter_context(tc.tile_pool(name="attn_ps", bufs=2, space="PSUM"))
        gates_ps = ctx.enter_context(tc.tile_pool(name="gates_ps", bufs=2, space="PSUM"))
        pre_ps = ctx.enter_context(tc.tile_pool(name="pre_ps", bufs=2, space="PSUM"))

        # ---- static weights into SBUF ----
        WG = wpool.tile([128, 80 * 128], bf16, tag="WG")
        nc.sync.dma_start(WG[:], wg_d[:])
        WQ3FT = wpool.tile([128, 512], f32, tag="WQ3FT")
        nc.sync.dma_start(WQ3FT[:], wq3f_d[:])
        MEMT = wpool.tile([128, 512], f32, tag="MEMT")
        nc.sync.dma_start(MEMT[:], memt_d[:])
        MEMBF = wpool.tile([128, 512], bf16, tag="MEMBF")
        nc.sync.dma_start(MEMBF[:], membf_d[:])
        WFCT = wpool.tile([128, 512], f32, tag="WFCT")
        nc.sync.dma_start(WFCT[:], wfct_d[:])
        WQZ = wpool.tile([128, 128], f32, tag="WQZ")
        nc.sync.dma_start(WQZ[:], wqz_d[:])
        WQZP = wpool.tile([128, 128], f32, tag="WQZP")
        nc.sync.dma_start(WQZP[:], wqzp_d[:])
        BIAST = wpool.tile([128, 16], f32, tag="BIAST")
        nc.sync.dma_start(BIAST[:], biast_d[:])
        SCAL = wpool.tile([128, 8], f32, tag="SCAL")
        nc.sync.dma_start(SCAL[:], scal_d[:])
        ONESF = wpool.tile([128, 128], bf16, tag="ONESF")
        nc.vector.memset(ONESF[:], 1.0)
        ONESC = wpool.tile([128, 1], bf16, tag="ONESC")
        nc.vector.memset(ONESC[:], 1.0)

        dgz, bgz = SCAL[:, 0:1], SCAL[:, 1:2]
        dgzp, bgzp = SCAL[:, 2:3], SCAL[:, 3:4]
        bq_ap, bfc_ap = SCAL[:, 4:5], SCAL[:, 5:6]

        # ---- persistent tiles ----
        ls_z = lszp.tile([128, TB], f32, tag="ls_z")
        Xpad = pers.tile([128, BB], bf16, tag="Xpad")
        nc.vector.memset(Xpad[:], 0.0)

        h_f = pers.tile([128, 128], f32, tag="h_f")
        h_b = pers.tile([128, 128], bf16, tag="h_b")
        c_t = pers.tile([128, 128], f32, tag="c_t")
        nc.vector.memset(h_f[:], 0.0)
        nc.vector.memset(h_b[:], 0.0)
        nc.vector.memset(c_t[:], 0.0)

        # ---- precompute z/zp and ls_z in T-chunks ----
        with nc.named_scope("precompute"):
            for cc in range(NTCH):
                sl = slice(cc * CCH, (cc + 1) * CCH)
                ch = {}
                for name in ["x", "xl", "mask", "delta", "xlb", "dltb", "xmb"]:
                    t_ = inp.tile([128, CCH], f32, tag=f"in_{name}")
                    nc.sync.dma_start(t_[:], dt_in[name][:, sl])
                    ch[name] = t_

                def zchain(dsrc, xlsrc, dg, bg, tag):
                    r1 = pre.tile([128, CCH], f32, tag="tA")
                    nc.scalar.activation(r1[:], dsrc[:], AF.Relu, scale=dg, bias=bg)
                    dz = pre.tile([128, CCH], f32, tag="tB")
                    nc.scalar.activation(dz[:], r1[:], AF.Exp, scale=-1.0)
                    u = pre.tile([128, CCH], f32, tag="tA")
                    nc.vector.tensor_tensor(u[:], xlsrc[:], ch["xmb"][:], ALU.subtract)
                    v = pre.tile([128, CCH], f32, tag="tB2")
                    nc.vector.tensor_tensor(v[:], dz[:], u[:], ALU.mult)
                    w = pre.tile([128, CCH], f32, tag="tC")
                    nc.vector.tensor_tensor(w[:], v[:], ch["xmb"][:], ALU.add)
                    d_ = pre.tile([128, CCH], f32, tag="tA")
                    nc.vector.tensor_tensor(d_[:], ch["x"][:], w[:], ALU.subtract)
                    e2 = pre.tile([128, CCH], f32, tag="tB")
                    nc.vector.tensor_tensor(e2[:], ch["mask"][:], d_[:], ALU.mult)
                    z_ = pre.tile([128, CCH], f32, tag=f"z{tag}")
                    nc.vector.tensor_tensor(z_[:], w[:], e2[:], ALU.add)
                    return z_

                z_c = zchain(ch["delta"], ch["xl"], dgz, bgz, "z")
                zp_c = zchain(ch["dltb"], ch["xlb"], dgzp, bgzp, "p")
                if dbg and cc == 0:
                    nc.sync.dma_start(dbg_d["z"][:], z_c[:, 0:BB])
                    nc.sync.dma_start(dbg_d["zp"][:], zp_c[:, 0:BB])

                for off in range(0, CCH, 512):
                    n = min(512, CCH - off)
                    pp = pre_ps.tile([128, 512], f32, tag="pp")
                    nc.tensor.matmul(pp[:, :n], lhsT=WQZ[:], rhs=z_c[:, off:off + n],
                                     start=True, stop=False)
                    nc.tensor.matmul(pp[:, :n], lhsT=WQZP[:], rhs=zp_c[:, off:off + n],
                                     start=False, stop=True)
                    nc.scalar.activation(ls_z[:, cc * CCH + off: cc * CCH + off + n],
                                         pp[:, :n], AF.Identity, bias=bq_ap)

        # ---- recurrence ----
        for t in range(T):
            with nc.named_scope(f"step{t}" if t % 10 == 0 else "step"):
                pa = attn_ps.tile([128, 512], f32, tag="pa")
                # ls = ls_z[t] + WQ3F.T @ h   (fp32)
                for k in range(4):
                    nc.tensor.matmul(pa[:, 0:32], lhsT=WQ3FT[:, 128 * k:128 * (k + 1)],
                                     rhs=h_f[:, 32 * k:32 * k + 32],
                                     start=(k == 0), stop=(k == 3))
                lsf = stp.tile([128, BB], f32, tag="lsf")
                nc.vector.tensor_tensor(lsf[:], pa[:, 0:32], ls_z[:, 32 * t:32 * t + 32], ALU.add)
                # logits^T = memory @ ls  (fp32), 4 M-chunks
                for j in range(4):
                    nc.tensor.matmul(pa[:, 128 + 32 * j:128 + 32 * (j + 1)],
                                     lhsT=MEMT[:, 128 * j:128 * (j + 1)], rhs=lsf[:],
                                     start=True, stop=True)
                eT = stp.tile([128, 128], bf16, tag="eT")
                nc.scalar.activation(eT[:], pa[:, 128:256], AF.Exp)
                # sums over M (partition dim) via ones matmul -> [1, 128]
                nc.tensor.matmul(pa[0:1, 320:448], lhsT=ONESC[:], rhs=eT[:],
                                 start=True, stop=True)
                sums = stp.tile([1, BB], f32, tag="sums")
                nc.vector.tensor_reduce(sums[:], pa[0:1, 320:448].rearrange("p (c b) -> p b c", c=4),
                                        axis=mybir.AxisListType.X, op=ALU.add)
                recipf = stp.tile([1, BB], f32, tag="recipf")
                nc.vector.reciprocal(recipf[:], sums[:])
                nc.vector.tensor_copy(Xpad[0:1, :], recipf[:])
                # gd^T = memory.T-chunks @ e^T  (bf16)
                for j in range(4):
                    nc.tensor.matmul(pa[:, 256:288], lhsT=MEMBF[:, 128 * j:128 * (j + 1)],
                                     rhs=eT[:, 32 * j:32 * j + 32],
                                     start=(j == 0), stop=(j == 3))
                # broadcast recip over partitions: ones[128,128].T @ Xpad
                nc.tensor.matmul(pa[:, 288:320], lhsT=ONESF[:], rhs=Xpad[:],
                                 start=True, stop=True)
                s_sb = stp.tile([128, BB], f32, tag="s_sb")
                nc.scalar.activation(s_sb[:], pa[:, 288:320], AF.Identity)
                gdn = stp.tile([128, BB], bf16, tag="gdn")
                nc.vector.tensor_tensor(gdn[:], pa[:, 256:288], s_sb[:], ALU.mult)
                # gates: per-chunk contiguous groups [ih, hh x4]
                pg = gates_ps.tile([128, 512], f32, tag="pg")
                for g in range(16):
                    nc.tensor.matmul(pg[:, 32 * g:32 * g + 32],
                                     lhsT=WG[:, 128 * (g * 5):128 * (g * 5 + 1)],
                                     rhs=gdn[:], start=True, stop=False)
                    for k in range(4):
                        nc.tensor.matmul(pg[:, 32 * g:32 * g + 32],
                                         lhsT=WG[:, 128 * (g * 5 + 1 + k):128 * (g * 5 + 2 + k)],
                                         rhs=h_b[:, 32 * k:32 * k + 32],
                                         start=False, stop=(k == 3))
                # pointwise: Y = tanh(scale*gates + bias')
                Y = stp.tile([128, 512], f32, tag="Y")
                for g in range(16):
                    sc = 1.0 if g // 4 == 2 else 0.5
                    nc.scalar.activation(Y[:, 32 * g:32 * g + 32], pg[:, 32 * g:32 * g + 32],
                                         AF.Tanh, scale=sc, bias=BIAST[:, g:g + 1])
                SI = stp.tile([128, 128], f32, tag="SI")
                nc.vector.tensor_scalar(SI[:], Y[:, 0:128], 1.0, 0.5, ALU.add, ALU.mult)
                SF = stp.tile([128, 128], f32, tag="SF")
                nc.vector.tensor_scalar(SF[:], Y[:, 128:256], 1.0, 0.5, ALU.add, ALU.mult)
                SO = stp.tile([128, 128], f32, tag="SO")
                nc.vector.tensor_scalar(SO[:], Y[:, 384:512], 1.0, 0.5, ALU.add, ALU.mult)
                m1 = stp.tile([128, 128], f32, tag="m1")
                nc.vector.tensor_tensor(m1[:], SF[:], c_t[:], ALU.mult)
                m2 = stp.tile([128, 128], f32, tag="m2")
                nc.vector.tensor_tensor(m2[:], SI[:], Y[:, 256:384], ALU.mult)
                c_new = state.tile([128, 128], f32, tag="c_t2")
                nc.vector.tensor_tensor(c_new[:], m1[:], m2[:], ALU.add)
                TC = stp.tile([128, 128], f32, tag="TC")
                nc.scalar.activation(TC[:], c_new[:], AF.Tanh)
                h_new = state.tile([128, 128], f32, tag="h_f2")
                nc.vector.tensor_tensor(h_new[:], SO[:], TC[:], ALU.mult)
                hb_new = state.tile([128, 128], bf16, tag="h_b2")
                nc.vector.tensor_copy(hb_new[:], h_new[:])
                if dbg and t == int(os.environ.get('LG_DBGT', '0')):
                    Gd = stp.tile([128, 512], f32, tag="Gd")
                    nc.scalar.activation(Gd[:], pg[:], AF.Identity)
                    nc.sync.dma_start(dbg_d["G"][:], Gd[:])
                    hbf = stp.tile([128, 128], f32, tag="hbf")
                    nc.vector.tensor_copy(hbf[:], h_b[:])
                    nc.sync.dma_start(dbg_d["hbin"][:], hbf[:])
                    nc.sync.dma_start(dbg_d["lsz"][:], ls_z[:, 0:BB])
                    nc.sync.dma_start(dbg_d["lsf"][:], lsf[:])
                    eTf = stp.tile([128, 128], f32, tag="eTf")
                    nc.vector.tensor_copy(eTf[:], eT[:])
                    nc.sync.dma_start(dbg_d["eT"][:], eTf[:])
                    nc.sync.dma_start(dbg_d["ssb"][:], s_sb[:])
                    gdnf = stp.tile([128, BB], f32, tag="gdnf")
                    nc.vector.tensor_copy(gdnf[:], gdn[:])
                    nc.sync.dma_start(dbg_d["gdn"][:], gdnf[:])
                    nc.sync.dma_start(dbg_d["Y"][:], Y[:])
                    nc.sync.dma_start(dbg_d["h"][:], h_new[:])
                h_f, h_b, c_t = h_new, hb_new, c_new

        # ---- final output: out^T = W_fc @ h + b_fc ----
        with nc.named_scope("final"):
            pf = attn_ps.tile([128, 512], f32, tag="pa")
            for k in range(4):
                nc.tensor.matmul(pf[:, 0:32], lhsT=WFCT[:, 128 * k:128 * (k + 1)],
                                 rhs=h_f[:, 32 * k:32 * k + 32],
                                 start=(k == 0), stop=(k == 3))
            outt = stp.tile([O, BB], f32, tag="outt")
            nc.scalar.activation(outt[:], pf[:, 0:32], AF.Identity, bias=bfc_ap)
            nc.sync.dma_start(o_d[:], outt[:])

    nc.compile()
    return nc


def _prep_host(inputs):
    """Host-side: fold weights, build per-core input maps."""
    inp = {k: np.asarray(v, np.float32) for k, v in inputs.items()}
    dgz = np.ascontiguousarray(np.diag(inp["W_gz"]))
    dgzp = np.ascontiguousarray(np.diag(inp["W_gzp"]))
    Wq = inp["W_q"]
    WQ3F = (Wq[:, 2 * F:] @ inp["W_fc"]).astype(np.float32)       # [F, H]
    b_q_eff = (inp["b_q"] + Wq[:, 2 * F:] @ inp["b_fc"]).astype(np.float32)
    bias_g = (inp["b_ih"] + inp["b_hh"]).astype(np.float32)       # [2048]

    # gates weights: Wcat.T tiles; tile (g, k): k=0 -> W_ih cols, k=1..4 -> W_hh
    WcatT = np.concatenate([inp["W_ih"], inp["W_hh"]], axis=1).T  # [640, 2048]
    wg = np.empty((128, 80 * 128), np.float32)
    for g in range(16):
        for k in range(5):
            wg[:, 128 * (g * 5 + k):128 * (g * 5 + k + 1)] = \
                WcatT[128 * k:128 * (k + 1), 128 * g:128 * (g + 1)]

    wq3f = np.empty((128, 512), np.float32)    # (WQ3F.T) chunks [128hk, F]
    for k in range(4):
        wq3f[:, 128 * k:128 * (k + 1)] = WQ3F.T[128 * k:128 * (k + 1), :]
    memt = np.ascontiguousarray(inp["memory"].T)                  # [F, M] = [128, 512]
    membf = np.empty((128, 512), np.float32)   # memory row-chunks [m_local, F]
    for j in range(4):
        membf[:, 128 * j:128 * (j + 1)] = inp["memory"][128 * j:128 * (j + 1), :]
    wfct = np.empty((128, 512), np.float32)    # W_fc.T chunks [128hk, O]
    for k in range(4):
        wfct[:, 128 * k:128 * (k + 1)] = inp["W_fc"].T[128 * k:128 * (k + 1), :]
    wqz = np.ascontiguousarray(Wq[:, 0:128].T)
    wqzp = np.ascontiguousarray(Wq[:, 128:256].T)

    biast = np.empty((128, 16), np.float32)
    for g in range(16):
        sc = 1.0 if g // 4 == 2 else 0.5
        biast[:, g] = bias_g[128 * g:128 * (g + 1)] * sc

    scal = np.zeros((128, 8), np.float32)
    scal[:, 0], scal[:, 1] = dgz, inp["b_gz"]
    scal[:, 2], scal[:, 3] = dgzp, inp["b_gzp"]
    scal[:, 4], scal[:, 5] = b_q_eff, inp["b_fc"]

    import ml_dtypes
    wg = wg.astype(ml_dtypes.bfloat16)
    membf = membf.astype(ml_dtypes.bfloat16)
    shared = dict(wg=wg, wq3f=wq3f, memt=memt, membf=membf, wfct=wfct,
                  wqz=wqz, wqzp=wqzp, biast=biast, scal=scal)

    xm_rep = np.ascontiguousarray(
        np.repeat(inp["X_mean"][:T].T[:, :, None], BB, axis=2).reshape(F, TB))
    in_maps = []
    ch_names = ["x", "xl", "mask", "delta", "xlb", "dltb"]
    ch_idx = [0, 1, 2, 3, 4, 5]
    for core in range(NC):
        b0 = core * BB
        m = dict(shared)
        sl = inp["input"][b0:b0 + BB]          # [BB, 6, 100, F]
        for nm, ci in zip(ch_names, ch_idx):
            # [F, T, BB] -> [F, T*BB]
            m[nm] = np.ascontiguousarray(
                np.transpose(sl[:, ci, :T], (2, 1, 0)).reshape(F, TB))
        m["xmb"] = xm_rep
        in_maps.append(m)
    return in_maps


def kernel(**inputs):
    global _built
    from concourse import bass_utils
    if _built is None:
        _built = _build()
    in_maps = _prep_host(inputs)
    res = bass_utils.run_bass_kernel_spmd(_built, in_maps, core_ids=list(range(NC)))
    out = np.empty((B, 1, O), np.float32)
    for core in range(NC):
        out[core * BB:(core + 1) * BB, 0, :] = res.results[core]["o"].T
    return out



# revision 5
# speedup vs baseline: 1.7531x; 1.7531x over previous
"""Trainium2 Bass kernel for nn_LGnet (LSTM + memory attention recurrence).

Sharding: data-parallel over batch, B=256 -> 32 rows per core across 8 cores.

Redesign vs baseline (all matmuls 16-bit, minimal critical chain):
  - logits computed DIRECTLY: logits^T = MW @ H + logz[t], where
    MW = 0.5*memory@(Wq3@W_fc) (host-folded) and logz[t] (z/zp gating part)
    is precomputed on-device and injected into PSUM via identity-rhs matmuls.
  - softmax denominator via broadcast-sums trick (all-ones stationary ->
    per-partition-replicated sums), one reciprocal, one multiply.
  - LSTM state kept as H=2h, D=2c; i/f/o rows of weights+bias host-scaled
    by 0.5 so the whole gate nonlinearity is ONE tanh table (exp+tanh share
    an ACT table set; sigmoid does not), pointwise is 4 fused
    scalar_tensor_tensor ops:  m1=(Yf+1)*D; m2=(Yi+1)*Yg; D'=0.5*m1+m2;
    H'=(Yo+1)*tanh(0.5*D').
  - W_hh matmul stream (64 MMs) overlaps the softmax scalar chain; o-gate
    chunks go to a separate PSUM bank and are computed last so the c-chain
    overlaps the remaining MMs.
  - gate bias injected into PSUM by [16,128]x[16,*] matmuls.
  - fp16 for weights/streams/state (3 extra mantissa bits vs bf16);
    bf16 only for exp outputs / reciprocal (range).
"""
import os
import numpy as np
from contextlib import ExitStack

B, T, F, H, O, M = 256, 100, 128, 512, 128, 512
T = int(os.environ.get("LG_T", str(T)))   # debug override; harness uses 100
assert T % 4 == 0
NC = 8
BB = B // NC          # 32 batch rows per core
TB = T * BB           # (t, b) packed columns
NSL = TB // 128       # 128-col slices (= T/4 tgroups)
SLPC = 5 if NSL % 5 == 0 else 1   # slices per precompute chunk
NTCH = NSL // SLPC
CCH = SLPC * 128      # cols per chunk

_built = None


def _build():
    import concourse.bass as bass
    import concourse.tile as tile
    from concourse import bacc, mybir

    f32 = mybir.dt.float32
    bf16 = mybir.dt.bfloat16
    f16 = mybir.dt.float16
    AF = mybir.ActivationFunctionType
    ALU = mybir.AluOpType
    nc = bacc.Bacc("TRN2", target_bir_lowering=False, debug=False, num_devices=NC)

    # ---- DRAM tensors (per-core data fed via in_maps) ----
    dt_in = {}
    for name in ["x", "xl", "mask", "delta", "xlb", "dltb", "xmb"]:
        dt_in[name] = nc.dram_tensor(name, [F, TB], f16, kind="ExternalInput").ap()
    mw_d = nc.dram_tensor("mw", [128, 16 * 128], f16, kind="ExternalInput").ap()
    whh_d = nc.dram_tensor("whh", [128, 64 * 128], f16, kind="ExternalInput").ap()
    wih_d = nc.dram_tensor("wih", [128, 16 * 128], f16, kind="ExternalInput").ap()
    membf_d = nc.dram_tensor("membf", [128, 512], bf16, kind="ExternalInput").ap()
    rz_d = nc.dram_tensor("rz", [128, 512], f16, kind="ExternalInput").ap()
    rzp_d = nc.dram_tensor("rzp", [128, 512], f16, kind="ExternalInput").ap()
    wfct_d = nc.dram_tensor("wfct", [128, 512], f16, kind="ExternalInput").ap()
    bias16_d = nc.dram_tensor("bias16", [16, 128], f16, kind="ExternalInput").ap()
    ind_d = nc.dram_tensor("ind", [16, 512], f16, kind="ExternalInput").ap()
    i32sel_d = nc.dram_tensor("i32sel", [128, 128], f16, kind="ExternalInput").ap()
    mbq1_d = nc.dram_tensor("mbq1", [1, 512], f16, kind="ExternalInput").ap()
    bfc1_d = nc.dram_tensor("bfc1", [1, 128], f16, kind="ExternalInput").ap()
    scal_d = nc.dram_tensor("scal", [128, 8], f32, kind="ExternalInput").ap()
    o_d = nc.dram_tensor("o", [O, BB], f32, kind="ExternalOutput").ap()

    with tile.TileContext(nc) as tc, ExitStack() as ctx:
        wpool = ctx.enter_context(tc.tile_pool(name="wpool", bufs=1))
        inp = ctx.enter_context(tc.tile_pool(name="inp", bufs=2))
        pre = ctx.enter_context(tc.tile_pool(name="pre", bufs=2))
        stp = ctx.enter_context(tc.tile_pool(name="stp", bufs=2))
        state = ctx.enter_context(tc.tile_pool(name="state", bufs=2))
        pers = ctx.enter_context(tc.tile_pool(name="pers", bufs=1))
        attn_ps = ctx.enter_context(tc.tile_pool(name="attn_ps", bufs=2, space="PSUM"))
        gi_ps = ctx.enter_context(tc.tile_pool(name="gi_ps", bufs=2, space="PSUM"))
        go_ps = ctx.enter_context(tc.tile_pool(name="go_ps", bufs=2, space="PSUM"))
        pre_ps = ctx.enter_context(tc.tile_pool(name="pre_ps", bufs=2, space="PSUM"))

        # ---- static weights into SBUF ----
        def wload(name, shape, dt_, src):
            t_ = wpool.tile(shape, dt_, tag=name)
            nc.sync.dma_start(t_[:], src[:])
            return t_
        MW = wload("MW", [128, 16 * 128], f16, mw_d)
        WHH = wload("WHH", [128, 64 * 128], f16, whh_d)
        WIH = wload("WIH", [128, 16 * 128], f16, wih_d)
        MEMBF = wload("MEMBF", [128, 512], bf16, membf_d)
        RZ = wload("RZ", [128, 512], f16, rz_d)
        RZP = wload("RZP", [128, 512], f16, rzp_d)
        WFCT = wload("WFCT", [128, 512], f16, wfct_d)
        BIAS16 = wload("BIAS16", [16, 128], f16, bias16_d)
        IND = wload("IND", [16, 512], f16, ind_d)
        I32S = wload("I32S", [128, 128], f16, i32sel_d)
        MBQ1 = wload("MBQ1", [1, 512], f16, mbq1_d)
        BFC1 = wload("BFC1", [1, 128], f16, bfc1_d)
        SCAL = wload("SCAL", [128, 8], f32, scal_d)
        ONESB = wpool.tile([128, 128], bf16, tag="ONESB")
        nc.vector.memset(ONESB[:], 1.0)
        ONES1 = wpool.tile([1, 128], f16, tag="ONES1")
        nc.vector.memset(ONES1[:], 1.0)
        ONES32 = wpool.tile([1, 32], f16, tag="ONES32")
        nc.vector.memset(ONES32[:], 1.0)

        ndgz, nbgz = SCAL[:, 0:1], SCAL[:, 1:2]
        ndgzp, nbgzp = SCAL[:, 2:3], SCAL[:, 3:4]

        # ---- persistent tiles ----
        z_t = wpool.tile([128, TB], f16, tag="z_t")
        zp_t = wpool.tile([128, TB], f16, tag="zp_t")
        logz = wpool.tile([128, NSL * 512], f16, tag="logz")

        HT = pers.tile([128, 128], f16, tag="HT")
        Dt = pers.tile([128, 128], f32, tag="Dt")
        nc.vector.memset(HT[:], 0.0)
        nc.vector.memset(Dt[:], 0.0)

        # ---- precompute z/zp and logz in chunks ----
        with nc.named_scope("precompute"):
            for cc in range(NTCH):
                sl = slice(cc * CCH, (cc + 1) * CCH)
                ch = {}
                for name in ["x", "xl", "mask", "delta", "xlb", "dltb", "xmb"]:
                    t_ = inp.tile([128, CCH], f16, tag=f"in_{name}")
                    nc.sync.dma_start(t_[:], dt_in[name][:, sl])
                    ch[name] = t_

                def zchain(eng, dsrc, xlsrc, nscale, nbias, zdst, sfx):
                    # dz = min(exp(-(scale*d + bias)), 1)
                    dzf = pre.tile([128, CCH], f16, tag=f"dzf{sfx}")
                    nc.scalar.activation(dzf[:], dsrc[:], AF.Exp, scale=nscale, bias=nbias)
                    dz = pre.tile([128, CCH], f16, tag=f"dz{sfx}")
                    eng.tensor_scalar_min(dz[:], dzf[:], 1.0)
                    u = pre.tile([128, CCH], f16, tag=f"u{sfx}")
                    eng.tensor_tensor(u[:], xlsrc[:], ch["xmb"][:], ALU.subtract)
                    v = pre.tile([128, CCH], f16, tag=f"v{sfx}")
                    eng.tensor_tensor(v[:], dz[:], u[:], ALU.mult)
                    w = pre.tile([128, CCH], f16, tag=f"w{sfx}")
                    eng.tensor_tensor(w[:], v[:], ch["xmb"][:], ALU.add)
                    d_ = pre.tile([128, CCH], f16, tag=f"d{sfx}")
                    eng.tensor_tensor(d_[:], ch["x"][:], w[:], ALU.subtract)
                    e2 = pre.tile([128, CCH], f16, tag=f"e{sfx}")
                    eng.tensor_tensor(e2[:], ch["mask"][:], d_[:], ALU.mult)
                    eng.tensor_tensor(zdst[:, sl], w[:], e2[:], ALU.add)

                zchain(nc.vector, ch["delta"], ch["xl"], ndgz, nbgz, z_t, "z")
                zchain(nc.gpsimd, ch["dltb"], ch["xlb"], ndgzp, nbgzp, zp_t, "p")

                for s in range(cc * SLPC, (cc + 1) * SLPC):
                    pp = pre_ps.tile([128, 512], f32, tag="pp")
                    nc.tensor.matmul(pp[:], lhsT=ONES1[:], rhs=MBQ1[:],
                                     start=True, stop=False)
                    nc.tensor.matmul(pp[:], lhsT=z_t[:, 128 * s:128 * (s + 1)],
                                     rhs=RZ[:], start=False, stop=False)
                    nc.tensor.matmul(pp[:], lhsT=zp_t[:, 128 * s:128 * (s + 1)],
                                     rhs=RZP[:], start=False, stop=True)
                    nc.scalar.activation(logz[:, 512 * s:512 * (s + 1)], pp[:],
                                         AF.Identity)

        # ---- recurrence ----
        for t in range(T):
            tg, tl = t // 4, t % 4
            with nc.named_scope(f"step{t}" if t % 10 == 0 else "step"):
                pa = attn_ps.tile([128, 512], f32, tag="pa")
                pgi = gi_ps.tile([128, 384], f32, tag="pgi")
                pgo = go_ps.tile([128, 128], f32, tag="pgo")

                # gate bias into psum (constants only; runs during prev tail)
                nc.tensor.matmul(pgi[:], lhsT=BIAS16[:], rhs=IND[:, 0:384],
                                 start=True, stop=False)
                nc.tensor.matmul(pgo[:], lhsT=BIAS16[:], rhs=IND[:, 384:512],
                                 start=True, stop=False)
                # logits: one sequential accumulation group per 32-col slice
                for j in range(4):
                    nc.tensor.matmul(pa[:, 32 * j:32 * (j + 1)],
                                     lhsT=logz[:, 512 * tg + 128 * j:512 * tg + 128 * (j + 1)],
                                     rhs=I32S[:, 32 * tl:32 * tl + 32],
                                     start=True, stop=False)
                    for k in range(4):
                        nc.tensor.matmul(pa[:, 32 * j:32 * (j + 1)],
                                         lhsT=MW[:, 128 * (4 * j + k):128 * (4 * j + k + 1)],
                                         rhs=HT[:, 32 * k:32 * k + 32],
                                         start=False, stop=(k == 3))
                # e^T = exp(logits^T)  [128 m-part, (chunk, b)]
                ET = stp.tile([128, 128], bf16, tag="ET")
                nc.scalar.activation(ET[:], pa[:, 0:128], AF.Exp)

                def whh_block(glo, ghi):
                    for g in range(glo, ghi):
                        dst = pgi if g < 12 else pgo
                        off = 32 * g if g < 12 else 32 * (g - 12)
                        for k in range(4):
                            nc.tensor.matmul(dst[:, off:off + 32],
                                             lhsT=WHH[:, 128 * (4 * g + k):128 * (4 * g + k + 1)],
                                             rhs=HT[:, 32 * k:32 * k + 32],
                                             start=False, stop=False)

                whh_block(0, 4)       # i chunks (16 MMs) while exp runs
                # sums bcast over partitions: pa[:,128:160] = sum_m e
                for c in range(4):
                    nc.tensor.matmul(pa[:, 128:160], lhsT=ONESB[:],
                                     rhs=ET[:, 32 * c:32 * c + 32],
                                     start=(c == 0), stop=(c == 3))
                # gd^T = memory^T-chunks @ e^T
                for j in range(4):
                    nc.tensor.matmul(pa[:, 160:192],
                                     lhsT=MEMBF[:, 128 * j:128 * (j + 1)],
                                     rhs=ET[:, 32 * j:32 * j + 32],
                                     start=(j == 0), stop=(j == 3))
                recipB = stp.tile([128, 32], bf16, tag="recipB")
                with nc.allow_low_precision(reason="softmax recip in bf16 is fine"):
                    nc.vector.reciprocal(recipB[:], pa[:, 128:160])
                GDN = stp.tile([128, 32], f16, tag="GDN")
                nc.vector.tensor_tensor(GDN[:], pa[:, 160:192], recipB[:], ALU.mult)

                whh_block(4, 12)      # f, g chunks (32 MMs) while recip/gdn run
                # W_ih @ gdn for i, f, g chunks
                for g in range(12):
                    nc.tensor.matmul(pgi[:, 32 * g:32 * (g + 1)],
                                     lhsT=WIH[:, 128 * g:128 * (g + 1)],
                                     rhs=GDN[:], start=False, stop=(g == 11))
                whh_block(12, 16)     # o chunks (16 MMs) while tanh/c-chain run
                for g in range(12, 16):
                    nc.tensor.matmul(pgo[:, 32 * (g - 12):32 * (g - 11)],
                                     lhsT=WIH[:, 128 * g:128 * (g + 1)],
                                     rhs=GDN[:], start=False, stop=(g == 15))

                # pointwise: Y = tanh(gates_scaled)
                Yifg = stp.tile([128, 384], f32, tag="Yifg")
                nc.scalar.activation(Yifg[:], pgi[:], AF.Tanh)
                Yo = stp.tile([128, 128], f32, tag="Yo")
                nc.scalar.activation(Yo[:], pgo[:], AF.Tanh)
                m1 = stp.tile([128, 128], f32, tag="m1")
                nc.vector.scalar_tensor_tensor(m1[:], Yifg[:, 128:256], 1.0, Dt[:],
                                               ALU.add, ALU.mult)
                # (TensorScalarPtr is not legal on Pool; keep m2 on DVE but
                #  compute the (Yi+1) half on Pool so the two run in parallel)
                Yip1 = stp.tile([128, 128], f32, tag="Yip1")
                nc.gpsimd.tensor_scalar_add(Yip1[:], Yifg[:, 0:128], 1.0)
                m2 = stp.tile([128, 128], f32, tag="m2")
                nc.vector.tensor_tensor(m2[:], Yip1[:], Yifg[:, 256:384], ALU.mult)
                Dn = state.tile([128, 128], f32, tag="Dn")
                nc.vector.scalar_tensor_tensor(Dn[:], m1[:], 0.5, m2[:],
                                               ALU.mult, ALU.add)
                TC = stp.tile([128, 128], f32, tag="TC")
                nc.scalar.activation(TC[:], Dn[:], AF.Tanh, scale=0.5)
                Hn = state.tile([128, 128], f16, tag="Hn")
                nc.vector.scalar_tensor_tensor(Hn[:], Yo[:], 1.0, TC[:],
                                               ALU.add, ALU.mult)
                HT, Dt = Hn, Dn

        # ---- final output: out^T = 0.5*W_fc @ H + b_fc ----
        with nc.named_scope("final"):
            pf = attn_ps.tile([128, 512], f32, tag="pa")
            nc.tensor.matmul(pf[:, 0:32], lhsT=BFC1[:], rhs=ONES32[:],
                             start=True, stop=False)
            for k in range(4):
                nc.tensor.matmul(pf[:, 0:32], lhsT=WFCT[:, 128 * k:128 * (k + 1)],
                                 rhs=HT[:, 32 * k:32 * k + 32],
                                 start=False, stop=(k == 3))
            outt = stp.tile([O, BB], f32, tag="outt")
            nc.scalar.activation(outt[:], pf[:, 0:32], AF.Identity)
            nc.sync.dma_start(o_d[:], outt[:])

    nc.compile()
    return nc


def _prep_host(inputs):
    """Host-side: fold weights, build per-core input maps."""
    import ml_dtypes
    b16 = ml_dtypes.bfloat16
    f16 = np.float16
    inp = {k: np.asarray(v, np.float32) for k, v in inputs.items()}
    mem = inp["memory"]
    Wq = inp["W_q"]
    Wq1, Wq2, Wq3 = Wq[:, :F], Wq[:, F:2 * F], Wq[:, 2 * F:]

    # gate row scaling: 0.5 for i,f,o (tanh trick), 1.0 for g; W_hh also *0.5 (H=2h)
    r = np.full((4 * H, 1), 0.5, np.float32)
    r[2 * H:3 * H] = 1.0
    WIHs = r * inp["W_ih"]
    WHHs = r * inp["W_hh"] * 0.5
    biass = r[:, 0] * (inp["b_ih"] + inp["b_hh"])

    wih = np.empty((128, 16 * 128), np.float32)
    for g in range(16):
        wih[:, 128 * g:128 * (g + 1)] = WIHs[128 * g:128 * (g + 1), :].T
    whh = np.empty((128, 64 * 128), np.float32)
    for g in range(16):
        for k in range(4):
            whh[:, 128 * (4 * g + k):128 * (4 * g + k + 1)] = \
                WHHs[128 * g:128 * (g + 1), 128 * k:128 * (k + 1)].T
    MWmat = 0.5 * (mem @ (Wq3 @ inp["W_fc"]))          # [M, H]
    mw = np.empty((128, 16 * 128), np.float32)
    for j in range(4):
        for k in range(4):
            mw[:, 128 * (4 * j + k):128 * (4 * j + k + 1)] = \
                MWmat[128 * j:128 * (j + 1), 128 * k:128 * (k + 1)].T
    membf = np.empty((128, 512), np.float32)
    for j in range(4):
        membf[:, 128 * j:128 * (j + 1)] = mem[128 * j:128 * (j + 1), :]
    wfct = np.empty((128, 512), np.float32)
    WFCs = (0.5 * inp["W_fc"]).T                       # [H, O]
    for k in range(4):
        wfct[:, 128 * k:128 * (k + 1)] = WFCs[128 * k:128 * (k + 1), :]

    bias16 = biass.reshape(16, 128)
    ind = np.zeros((16, 512), np.float32)
    for g in range(16):
        ind[g, 32 * g:32 * (g + 1)] = 1.0
    i32sel = np.eye(128, dtype=f16)
    mbq1 = (mem @ (inp["b_q"] + Wq3 @ inp["b_fc"]))[None, :]
    bfc1 = inp["b_fc"][None, :]

    scal = np.zeros((128, 8), np.float32)
    scal[:, 0] = -np.diag(inp["W_gz"])
    scal[:, 1] = -inp["b_gz"]
    scal[:, 2] = -np.diag(inp["W_gzp"])
    scal[:, 3] = -inp["b_gzp"]

    shared = dict(
        mw=mw.astype(f16), whh=whh.astype(f16), wih=wih.astype(f16),
        membf=membf.astype(b16), rz=(mem @ Wq1).T.astype(f16),
        rzp=(mem @ Wq2).T.astype(f16), wfct=wfct.astype(f16),
        bias16=bias16.astype(f16), ind=ind.astype(f16), i32sel=i32sel,
        mbq1=mbq1.astype(f16), bfc1=bfc1.astype(f16), scal=scal,
    )

    xm_rep = np.repeat(inp["X_mean"][:T].T[:, :, None], BB, axis=2).reshape(F, TB)
    xm_rep = np.ascontiguousarray(xm_rep).astype(f16)
    in_maps = []
    ch_names = ["x", "xl", "mask", "delta", "xlb", "dltb"]
    for core in range(NC):
        b0 = core * BB
        m_ = dict(shared)
        sl = inp["input"][b0:b0 + BB]          # [BB, 6, 100, F]
        for ci, nm in enumerate(ch_names):
            m_[nm] = np.ascontiguousarray(
                np.transpose(sl[:, ci, :T], (2, 1, 0)).reshape(F, TB)).astype(f16)
        m_["xmb"] = xm_rep
        in_maps.append(m_)
    return in_maps


def kernel(**inputs):
    global _built
    from concourse import bass_utils
    if _built is None:
        _built = _build()
    in_maps = _prep_host(inputs)
    res = bass_utils.run_bass_kernel_spmd(_built, in_maps, core_ids=list(range(NC)))
    out = np.empty((B, 1, O), np.float32)
    for core in range(NC):
        out[core * BB:(core + 1) * BB, 0, :] = res.results[core]["o"].T
    return out


# revision 7
# speedup vs baseline: 2.3872x; 1.3617x over previous
"""Trainium2 Bass kernel for nn_LGnet (LSTM + memory attention recurrence).

Sharding: data-parallel over batch, B=256 -> 32 rows per core across 8 cores.

Redesign vs baseline (all matmuls 16-bit, minimal critical chain):
  - logits computed DIRECTLY: logits^T = MW @ H + logz[t], where
    MW = 0.5*memory@(Wq3@W_fc) (host-folded) and logz[t] (z/zp gating part)
    is precomputed on-device and injected into PSUM via identity-rhs matmuls.
  - softmax denominator via broadcast-sums trick (all-ones stationary ->
    per-partition-replicated sums), one reciprocal, one multiply.
  - LSTM state kept as H=2h, D=2c; i/f/o rows of weights+bias host-scaled
    by 0.5 so the whole gate nonlinearity is ONE tanh table (exp+tanh share
    an ACT table set; sigmoid does not), pointwise is 4 fused
    scalar_tensor_tensor ops:  m1=(Yf+1)*D; m2=(Yi+1)*Yg; D'=0.5*m1+m2;
    H'=(Yo+1)*tanh(0.5*D').
  - W_hh matmul stream (64 MMs) overlaps the softmax scalar chain; o-gate
    chunks go to a separate PSUM bank and are computed last so the c-chain
    overlaps the remaining MMs.
  - gate bias injected into PSUM by [16,128]x[16,*] matmuls.
  - fp16 for weights/streams/state (3 extra mantissa bits vs bf16);
    bf16 only for exp outputs / reciprocal (range).
"""
import os
import numpy as np
from contextlib import ExitStack

B, T, F, H, O, M = 256, 100, 128, 512, 128, 512
T = int(os.environ.get("LG_T", str(T)))   # debug override; harness uses 100
assert T % 4 == 0
NC = 8
BB = B // NC          # 32 batch rows per core
TB = T * BB           # (t, b) packed columns
NSL = TB // 128       # 128-col slices (= T/4 tgroups)
SLPC = 5 if NSL % 5 == 0 else 1   # slices per precompute chunk
NTCH = NSL // SLPC
CCH = SLPC * 128      # cols per chunk

_built = None


def _build():
    import concourse.bass as bass
    import concourse.tile as tile
    from concourse import bacc, mybir

    f32 = mybir.dt.float32
    bf16 = mybir.dt.bfloat16
    f16 = mybir.dt.float16
    AF = mybir.ActivationFunctionType
    ALU = mybir.AluOpType
    nc = bacc.Bacc("TRN2", target_bir_lowering=False, debug=False, num_devices=NC)

    # ---- DRAM tensors (per-core data fed via in_maps) ----
    dt_in = {}
    for name in ["x", "xl", "mask", "delta", "xlb", "dltb", "xmb"]:
        dt_in[name] = nc.dram_tensor(name, [F, TB], f16, kind="ExternalInput").ap()
    mw_d = nc.dram_tensor("mw", [128, 16 * 128], f16, kind="ExternalInput").ap()
    whh_d = nc.dram_tensor("whh", [128, 64 * 128], f16, kind="ExternalInput").ap()
    wih_d = nc.dram_tensor("wih", [128, 16 * 128], f16, kind="ExternalInput").ap()
    membf_d = nc.dram_tensor("membf", [128, 512], bf16, kind="ExternalInput").ap()
    rz_d = nc.dram_tensor("rz", [128, 512], f16, kind="ExternalInput").ap()
    rzp_d = nc.dram_tensor("rzp", [128, 512], f16, kind="ExternalInput").ap()
    wfct_d = nc.dram_tensor("wfct", [128, 512], f16, kind="ExternalInput").ap()
    bias16_d = nc.dram_tensor("bias16", [16, 128], f16, kind="ExternalInput").ap()
    ind_d = nc.dram_tensor("ind", [16, 512], f16, kind="ExternalInput").ap()
    i32sel_d = nc.dram_tensor("i32sel", [128, 128], f16, kind="ExternalInput").ap()
    mbq1_d = nc.dram_tensor("mbq1", [1, 512], f16, kind="ExternalInput").ap()
    bfc1_d = nc.dram_tensor("bfc1", [1, 128], f16, kind="ExternalInput").ap()
    scal_d = nc.dram_tensor("scal", [128, 8], f32, kind="ExternalInput").ap()
    o_d = nc.dram_tensor("o", [O, BB], f32, kind="ExternalOutput").ap()

    with tile.TileContext(nc) as tc, ExitStack() as ctx:
        wpool = ctx.enter_context(tc.tile_pool(name="wpool", bufs=1))
        inp = ctx.enter_context(tc.tile_pool(name="inp", bufs=2))
        pre = ctx.enter_context(tc.tile_pool(name="pre", bufs=2))
        stp = ctx.enter_context(tc.tile_pool(name="stp", bufs=2))
        state = ctx.enter_context(tc.tile_pool(name="state", bufs=2))
        pers = ctx.enter_context(tc.tile_pool(name="pers", bufs=1))
        attn_ps = ctx.enter_context(tc.tile_pool(name="attn_ps", bufs=2, space="PSUM"))
        gi_ps = ctx.enter_context(tc.tile_pool(name="gi_ps", bufs=2, space="PSUM"))
        go_ps = ctx.enter_context(tc.tile_pool(name="go_ps", bufs=2, space="PSUM"))
        pre_ps = ctx.enter_context(tc.tile_pool(name="pre_ps", bufs=2, space="PSUM"))

        # ---- static weights into SBUF ----
        def wload(name, shape, dt_, src):
            t_ = wpool.tile(shape, dt_, tag=name)
            nc.sync.dma_start(t_[:], src[:])
            return t_
        MW = wload("MW", [128, 16 * 128], f16, mw_d)
        WHH = wload("WHH", [128, 64 * 128], f16, whh_d)
        WIH = wload("WIH", [128, 16 * 128], f16, wih_d)
        MEMBF = wload("MEMBF", [128, 512], bf16, membf_d)
        RZ = wload("RZ", [128, 512], f16, rz_d)
        RZP = wload("RZP", [128, 512], f16, rzp_d)
        WFCT = wload("WFCT", [128, 512], f16, wfct_d)
        BIAS16 = wload("BIAS16", [16, 128], f16, bias16_d)
        IND = wload("IND", [16, 512], f16, ind_d)
        I32S = wload("I32S", [128, 128], f16, i32sel_d)
        MBQ1 = wload("MBQ1", [1, 512], f16, mbq1_d)
        BFC1 = wload("BFC1", [1, 128], f16, bfc1_d)
        SCAL = wload("SCAL", [128, 8], f32, scal_d)
        ONESB = wpool.tile([128, 128], bf16, tag="ONESB")
        nc.vector.memset(ONESB[:], 1.0)
        ONES1 = wpool.tile([1, 128], f16, tag="ONES1")
        nc.vector.memset(ONES1[:], 1.0)
        ONES32 = wpool.tile([1, 32], f16, tag="ONES32")
        nc.vector.memset(ONES32[:], 1.0)

        ndgz, nbgz = SCAL[:, 0:1], SCAL[:, 1:2]
        ndgzp, nbgzp = SCAL[:, 2:3], SCAL[:, 3:4]

        # ---- persistent tiles ----
        z_t = wpool.tile([128, TB], f16, tag="z_t")
        zp_t = wpool.tile([128, TB], f16, tag="zp_t")
        logz = wpool.tile([128, NSL * 512], f16, tag="logz")

        HT = pers.tile([128, 128], f16, tag="HT")
        Dt = pers.tile([128, 128], f32, tag="Dt")
        nc.vector.memset(HT[:], 0.0)
        nc.vector.memset(Dt[:], 0.0)

        # ---- precompute z/zp and logz in chunks ----
        with nc.named_scope("precompute"):
            for cc in range(NTCH):
                sl = slice(cc * CCH, (cc + 1) * CCH)
                ch = {}
                for name in ["x", "xl", "mask", "delta", "xlb", "dltb", "xmb"]:
                    t_ = inp.tile([128, CCH], f16, tag=f"in_{name}")
                    nc.sync.dma_start(t_[:], dt_in[name][:, sl])
                    ch[name] = t_

                def zchain(eng, dsrc, xlsrc, nscale, nbias, zdst, sfx):
                    # dz = min(exp(-(scale*d + bias)), 1)
                    dzf = pre.tile([128, CCH], f16, tag=f"dzf{sfx}")
                    nc.scalar.activation(dzf[:], dsrc[:], AF.Exp, scale=nscale, bias=nbias)
                    dz = pre.tile([128, CCH], f16, tag=f"dz{sfx}")
                    eng.tensor_scalar_min(dz[:], dzf[:], 1.0)
                    u = pre.tile([128, CCH], f16, tag=f"u{sfx}")
                    eng.tensor_tensor(u[:], xlsrc[:], ch["xmb"][:], ALU.subtract)
                    v = pre.tile([128, CCH], f16, tag=f"v{sfx}")
                    eng.tensor_tensor(v[:], dz[:], u[:], ALU.mult)
                    w = pre.tile([128, CCH], f16, tag=f"w{sfx}")
                    eng.tensor_tensor(w[:], v[:], ch["xmb"][:], ALU.add)
                    d_ = pre.tile([128, CCH], f16, tag=f"d{sfx}")
                    eng.tensor_tensor(d_[:], ch["x"][:], w[:], ALU.subtract)
                    e2 = pre.tile([128, CCH], f16, tag=f"e{sfx}")
                    eng.tensor_tensor(e2[:], ch["mask"][:], d_[:], ALU.mult)
                    eng.tensor_tensor(zdst[:, sl], w[:], e2[:], ALU.add)

                # (Pool/GpSimd tensor ops are ~10x slower than DVE on trn2 —
                #  keep all elementwise work on the vector engine)
                zchain(nc.vector, ch["delta"], ch["xl"], ndgz, nbgz, z_t, "z")
                zchain(nc.vector, ch["dltb"], ch["xlb"], ndgzp, nbgzp, zp_t, "p")

                for s in range(cc * SLPC, (cc + 1) * SLPC):
                    pp = pre_ps.tile([128, 512], f32, tag="pp")
                    nc.tensor.matmul(pp[:], lhsT=ONES1[:], rhs=MBQ1[:],
                                     start=True, stop=False)
                    nc.tensor.matmul(pp[:], lhsT=z_t[:, 128 * s:128 * (s + 1)],
                                     rhs=RZ[:], start=False, stop=False)
                    nc.tensor.matmul(pp[:], lhsT=zp_t[:, 128 * s:128 * (s + 1)],
                                     rhs=RZP[:], start=False, stop=True)
                    nc.scalar.activation(logz[:, 512 * s:512 * (s + 1)], pp[:],
                                         AF.Identity)

        # ---- recurrence ----
        for t in range(T):
            tg, tl = t // 4, t % 4
            with nc.named_scope(f"step{t}" if t % 10 == 0 else "step"):
                pa = attn_ps.tile([128, 512], f32, tag="pa")
                pgi = gi_ps.tile([128, 384], f32, tag="pgi")
                pgo = go_ps.tile([128, 128], f32, tag="pgo")

                # gate bias into psum (constants only; runs during prev tail)
                nc.tensor.matmul(pgi[:], lhsT=BIAS16[:], rhs=IND[:, 0:384],
                                 start=True, stop=False)
                nc.tensor.matmul(pgo[:], lhsT=BIAS16[:], rhs=IND[:, 384:512],
                                 start=True, stop=False)
                # logits: one sequential accumulation group per 32-col slice
                for j in range(4):
                    nc.tensor.matmul(pa[:, 32 * j:32 * (j + 1)],
                                     lhsT=logz[:, 512 * tg + 128 * j:512 * tg + 128 * (j + 1)],
                                     rhs=I32S[:, 32 * tl:32 * tl + 32],
                                     start=True, stop=False)
                    for k in range(4):
                        nc.tensor.matmul(pa[:, 32 * j:32 * (j + 1)],
                                         lhsT=MW[:, 128 * (4 * j + k):128 * (4 * j + k + 1)],
                                         rhs=HT[:, 32 * k:32 * k + 32],
                                         start=False, stop=(k == 3))
                # e^T = exp(logits^T)  [128 m-part, (chunk, b)]
                ET = stp.tile([128, 128], bf16, tag="ET")
                nc.scalar.activation(ET[:], pa[:, 0:128], AF.Exp)

                def whh_block(glo, ghi):
                    for g in range(glo, ghi):
                        dst = pgi if g < 12 else pgo
                        off = 32 * g if g < 12 else 32 * (g - 12)
                        for k in range(4):
                            nc.tensor.matmul(dst[:, off:off + 32],
                                             lhsT=WHH[:, 128 * (4 * g + k):128 * (4 * g + k + 1)],
                                             rhs=HT[:, 32 * k:32 * k + 32],
                                             start=False, stop=False)

                whh_block(0, 4)       # i chunks (16 MMs) while exp runs
                # sums bcast over partitions: pa[:,128:160] = sum_m e
                for c in range(4):
                    nc.tensor.matmul(pa[:, 128:160], lhsT=ONESB[:],
                                     rhs=ET[:, 32 * c:32 * c + 32],
                                     start=(c == 0), stop=(c == 3))
                # gd^T = memory^T-chunks @ e^T
                for j in range(4):
                    nc.tensor.matmul(pa[:, 160:192],
                                     lhsT=MEMBF[:, 128 * j:128 * (j + 1)],
                                     rhs=ET[:, 32 * j:32 * j + 32],
                                     start=(j == 0), stop=(j == 3))
                recipB = stp.tile([128, 32], bf16, tag="recipB")
                with nc.allow_low_precision(reason="softmax recip in bf16 is fine"):
                    nc.vector.reciprocal(recipB[:], pa[:, 128:160])
                GDN = stp.tile([128, 32], f16, tag="GDN")
                nc.vector.tensor_tensor(GDN[:], pa[:, 160:192], recipB[:], ALU.mult)

                whh_block(4, 12)      # f, g chunks (32 MMs) while recip/gdn run
                # W_ih @ gdn for i, f, g chunks
                for g in range(12):
                    nc.tensor.matmul(pgi[:, 32 * g:32 * (g + 1)],
                                     lhsT=WIH[:, 128 * g:128 * (g + 1)],
                                     rhs=GDN[:], start=False, stop=(g == 11))
                whh_block(12, 16)     # o chunks (16 MMs) while tanh/c-chain run
                for g in range(12, 16):
                    nc.tensor.matmul(pgo[:, 32 * (g - 12):32 * (g - 11)],
                                     lhsT=WIH[:, 128 * g:128 * (g + 1)],
                                     rhs=GDN[:], start=False, stop=(g == 15))

                # pointwise: Y = tanh(gates_scaled)
                Yifg = stp.tile([128, 384], f32, tag="Yifg")
                nc.scalar.activation(Yifg[:], pgi[:], AF.Tanh)
                Yo = stp.tile([128, 128], f32, tag="Yo")
                nc.scalar.activation(Yo[:], pgo[:], AF.Tanh)
                m1 = stp.tile([128, 128], f32, tag="m1")
                nc.vector.scalar_tensor_tensor(m1[:], Yifg[:, 128:256], 1.0, Dt[:],
                                               ALU.add, ALU.mult)
                m2 = stp.tile([128, 128], f32, tag="m2")
                nc.vector.scalar_tensor_tensor(m2[:], Yifg[:, 0:128], 1.0,
                                               Yifg[:, 256:384], ALU.add, ALU.mult)
                Dn = state.tile([128, 128], f32, tag="Dn")
                nc.vector.scalar_tensor_tensor(Dn[:], m1[:], 0.5, m2[:],
                                               ALU.mult, ALU.add)
                TC = stp.tile([128, 128], f32, tag="TC")
                nc.scalar.activation(TC[:], Dn[:], AF.Tanh, scale=0.5)
                Hn = state.tile([128, 128], f16, tag="Hn")
                nc.vector.scalar_tensor_tensor(Hn[:], Yo[:], 1.0, TC[:],
                                               ALU.add, ALU.mult)
                HT, Dt = Hn, Dn

        # ---- final output: out^T = 0.5*W_fc @ H + b_fc ----
        with nc.named_scope("final"):
            pf = attn_ps.tile([128, 512], f32, tag="pa")
            nc.tensor.matmul(pf[:, 0:32], lhsT=BFC1[:], rhs=ONES32[:],
                             start=True, stop=False)
            for k in range(4):
                nc.tensor.matmul(pf[:, 0:32], lhsT=WFCT[:, 128 * k:128 * (k + 1)],
                                 rhs=HT[:, 32 * k:32 * k + 32],
                                 start=False, stop=(k == 3))
            outt = stp.tile([O, BB], f32, tag="outt")
            nc.scalar.activation(outt[:], pf[:, 0:32], AF.Identity)
            nc.sync.dma_start(o_d[:], outt[:])

    nc.compile()
    return nc


def _prep_host(inputs):
    """Host-side: fold weights, build per-core input maps."""
    import ml_dtypes
    b16 = ml_dtypes.bfloat16
    f16 = np.float16
    inp = {k: np.asarray(v, np.float32) for k, v in inputs.items()}
    mem = inp["memory"]
    Wq = inp["W_q"]
    Wq1, Wq2, Wq3 = Wq[:, :F], Wq[:, F:2 * F], Wq[:, 2 * F:]

    # gate row scaling: 0.5 for i,f,o (tanh trick), 1.0 for g; W_hh also *0.5 (H=2h)
    r = np.full((4 * H, 1), 0.5, np.float32)
    r[2 * H:3 * H] = 1.0
    WIHs = r * inp["W_ih"]
    WHHs = r * inp["W_hh"] * 0.5
    biass = r[:, 0] * (inp["b_ih"] + inp["b_hh"])

    wih = np.empty((128, 16 * 128), np.float32)
    for g in range(16):
        wih[:, 128 * g:128 * (g + 1)] = WIHs[128 * g:128 * (g + 1), :].T
    whh = np.empty((128, 64 * 128), np.float32)
    for g in range(16):
        for k in range(4):
            whh[:, 128 * (4 * g + k):128 * (4 * g + k + 1)] = \
                WHHs[128 * g:128 * (g + 1), 128 * k:128 * (k + 1)].T
    MWmat = 0.5 * (mem @ (Wq3 @ inp["W_fc"]))          # [M, H]
    mw = np.empty((128, 16 * 128), np.float32)
    for j in range(4):
        for k in range(4):
            mw[:, 128 * (4 * j + k):128 * (4 * j + k + 1)] = \
                MWmat[128 * j:128 * (j + 1), 128 * k:128 * (k + 1)].T
    membf = np.empty((128, 512), np.float32)
    for j in range(4):
        membf[:, 128 * j:128 * (j + 1)] = mem[128 * j:128 * (j + 1), :]
    wfct = np.empty((128, 512), np.float32)
    WFCs = (0.5 * inp["W_fc"]).T                       # [H, O]
    for k in range(4):
        wfct[:, 128 * k:128 * (k + 1)] = WFCs[128 * k:128 * (k + 1), :]

    bias16 = biass.reshape(16, 128)
    ind = np.zeros((16, 512), np.float32)
    for g in range(16):
        ind[g, 32 * g:32 * (g + 1)] = 1.0
    i32sel = np.eye(128, dtype=f16)
    mbq1 = (mem @ (inp["b_q"] + Wq3 @ inp["b_fc"]))[None, :]
    bfc1 = inp["b_fc"][None, :]

    scal = np.zeros((128, 8), np.float32)
    scal[:, 0] = -np.diag(inp["W_gz"])
    scal[:, 1] = -inp["b_gz"]
    scal[:, 2] = -np.diag(inp["W_gzp"])
    scal[:, 3] = -inp["b_gzp"]

    shared = dict(
        mw=mw.astype(f16), whh=whh.astype(f16), wih=wih.astype(f16),
        membf=membf.astype(b16), rz=(mem @ Wq1).T.astype(f16),
        rzp=(mem @ Wq2).T.astype(f16), wfct=wfct.astype(f16),
        bias16=bias16.astype(f16), ind=ind.astype(f16), i32sel=i32sel,
        mbq1=mbq1.astype(f16), bfc1=bfc1.astype(f16), scal=scal,
    )

    xm_rep = np.repeat(inp["X_mean"][:T].T[:, :, None], BB, axis=2).reshape(F, TB)
    xm_rep = np.ascontiguousarray(xm_rep).astype(f16)
    in_maps = []
    ch_names = ["x", "xl", "mask", "delta", "xlb", "dltb"]
    for core in range(NC):
        b0 = core * BB
        m_ = dict(shared)
        sl = inp["input"][b0:b0 + BB]          # [BB, 6, 100, F]
        for ci, nm in enumerate(ch_names):
            m_[nm] = np.ascontiguousarray(
                np.transpose(sl[:, ci, :T], (2, 1, 0)).reshape(F, TB)).astype(f16)
        m_["xmb"] = xm_rep
        in_maps.append(m_)
    return in_maps


def kernel(**inputs):
    global _built
    from concourse import bass_utils
    if _built is None:
        _built = _build()
    in_maps = _prep_host(inputs)
    res = bass_utils.run_bass_kernel_spmd(_built, in_maps, core_ids=list(range(NC)))
    out = np.empty((B, 1, O), np.float32)
    for core in range(NC):
        out[core * BB:(core + 1) * BB, 0, :] = res.results[core]["o"].T
    return out


# revision 9
# speedup vs baseline: 2.4836x; 1.0404x over previous
"""Trainium2 Bass kernel for nn_LGnet (LSTM + memory attention recurrence).

Sharding: data-parallel over batch, B=256 -> 32 rows per core across 8 cores.

Design (all matmuls 16-bit, minimal critical chain):
  - z/zp gating streams are pure input preprocessing -> folded on HOST.
  - logits computed DIRECTLY: logits^T = MW @ H + logz[t], where
    MW = 0.5*memory@(Wq3@W_fc) (host-folded) and logz[t] = RZ.T@z + RZP.T@zp
    + mbq is precomputed on-device (matmuls; slices interleaved into the
    first steps) and injected into PSUM via identity-rhs matmuls.
  - softmax denominator via broadcast-sums trick (all-ones stationary ->
    per-partition-replicated sums), one reciprocal, one multiply.
  - LSTM state kept as H=2h, D=2c; i/f/o rows of weights+bias host-scaled
    by 0.5 so the whole gate nonlinearity is ONE tanh table (exp+tanh share
    an ACT table set; sigmoid does not), pointwise is 4 fused
    scalar_tensor_tensor ops:  m1=(Yf+1)*D; m2=(Yi+1)*Yg; D'=0.5*m1+m2;
    H'=(Yo+1)*tanh(0.5*D').
  - W_hh matmul stream overlaps the softmax scalar chain; sums/gd issued
    early so gdn is ready before the W_ih matmuls; o-gate chunks go to a
    separate PSUM bank and are computed last so the c-chain overlaps the
    remaining MMs.
  - gate bias injected into PSUM by [16,128]x[16,*] matmuls.
  - fp16 everywhere except exp outputs / reciprocal (bf16 for range).
"""
import os
import numpy as np
from contextlib import ExitStack

B, T, F, H, O, M = 256, 100, 128, 512, 128, 512
T = int(os.environ.get("LG_T", str(T)))   # debug override; harness uses 100
assert T % 4 == 0
NC = 8
BB = B // NC          # 32 batch rows per core
TB = T * BB           # (t, b) packed columns
NSL = TB // 128       # 128-col slices (= T/4 tgroups)

_built = None


def _build():
    import concourse.bass as bass
    import concourse.tile as tile
    from concourse import bacc, mybir

    f32 = mybir.dt.float32
    bf16 = mybir.dt.bfloat16
    f16 = mybir.dt.float16
    AF = mybir.ActivationFunctionType
    ALU = mybir.AluOpType
    nc = bacc.Bacc("TRN2", target_bir_lowering=False, debug=False, num_devices=NC)

    # ---- DRAM tensors (per-core data fed via in_maps) ----
    z_d = nc.dram_tensor("z", [F, TB], f16, kind="ExternalInput").ap()
    zp_d = nc.dram_tensor("zp", [F, TB], f16, kind="ExternalInput").ap()
    mw_d = nc.dram_tensor("mw", [128, 16 * 128], f16, kind="ExternalInput").ap()
    whh_d = nc.dram_tensor("whh", [128, 64 * 128], f16, kind="ExternalInput").ap()
    wih_d = nc.dram_tensor("wih", [128, 16 * 128], f16, kind="ExternalInput").ap()
    membf_d = nc.dram_tensor("membf", [128, 512], bf16, kind="ExternalInput").ap()
    rz_d = nc.dram_tensor("rz", [128, 512], f16, kind="ExternalInput").ap()
    rzp_d = nc.dram_tensor("rzp", [128, 512], f16, kind="ExternalInput").ap()
    wfct_d = nc.dram_tensor("wfct", [128, 512], f16, kind="ExternalInput").ap()
    bias16_d = nc.dram_tensor("bias16", [16, 128], f16, kind="ExternalInput").ap()
    ind_d = nc.dram_tensor("ind", [16, 512], f16, kind="ExternalInput").ap()
    i32sel_d = nc.dram_tensor("i32sel", [128, 128], f16, kind="ExternalInput").ap()
    mbqb_d = nc.dram_tensor("mbqb", [128, 512], f16, kind="ExternalInput").ap()
    bfc1_d = nc.dram_tensor("bfc1", [1, 128], f16, kind="ExternalInput").ap()
    o_d = nc.dram_tensor("o", [O, BB], f32, kind="ExternalOutput").ap()

    with tile.TileContext(nc) as tc, ExitStack() as ctx:
        wpool = ctx.enter_context(tc.tile_pool(name="wpool", bufs=1))
        stp = ctx.enter_context(tc.tile_pool(name="stp", bufs=2))
        state = ctx.enter_context(tc.tile_pool(name="state", bufs=2))
        pers = ctx.enter_context(tc.tile_pool(name="pers", bufs=1))
        attn_ps = ctx.enter_context(tc.tile_pool(name="attn_ps", bufs=2, space="PSUM"))
        gi_ps = ctx.enter_context(tc.tile_pool(name="gi_ps", bufs=2, space="PSUM"))
        go_ps = ctx.enter_context(tc.tile_pool(name="go_ps", bufs=2, space="PSUM"))
        pre_ps = ctx.enter_context(tc.tile_pool(name="pre_ps", bufs=2, space="PSUM"))

        # ---- static weights / inputs into SBUF ----
        def wload(name, shape, dt_, src):
            t_ = wpool.tile(shape, dt_, tag=name)
            nc.sync.dma_start(t_[:], src[:])
            return t_
        Z = wload("Z", [128, TB], f16, z_d)
        ZP = wload("ZP", [128, TB], f16, zp_d)
        MW = wload("MW", [128, 16 * 128], f16, mw_d)
        WHH = wload("WHH", [128, 64 * 128], f16, whh_d)
        WIH = wload("WIH", [128, 16 * 128], f16, wih_d)
        MEMBF = wload("MEMBF", [128, 512], bf16, membf_d)
        RZ = wload("RZ", [128, 512], f16, rz_d)
        RZP = wload("RZP", [128, 512], f16, rzp_d)
        WFCT = wload("WFCT", [128, 512], f16, wfct_d)
        BIAS16 = wload("BIAS16", [16, 128], f16, bias16_d)
        IND = wload("IND", [16, 512], f16, ind_d)
        I32S = wload("I32S", [128, 128], f16, i32sel_d)
        MBQB = wload("MBQB", [128, 512], f16, mbqb_d)
        BFC1 = wload("BFC1", [1, 128], f16, bfc1_d)
        ONESB = wpool.tile([128, 128], bf16, tag="ONESB")
        nc.vector.memset(ONESB[:], 1.0)
        ONES32 = wpool.tile([1, 32], f16, tag="ONES32")
        nc.vector.memset(ONES32[:], 1.0)

        logz = wpool.tile([128, NSL * 512], f16, tag="logz")

        HT = pers.tile([128, 128], f16, tag="HT")
        Dt = pers.tile([128, 128], f16, tag="Dt")
        nc.vector.memset(HT[:], 0.0)
        nc.vector.memset(Dt[:], 0.0)

        # logz slice s: [tb-part, m] = Z_s.T @ RZ + ZP_s.T @ RZP, + mbq via
        # a Pool-engine add during the PSUM->SBUF copy (Pool is otherwise idle).
        def logz_slice(s):
            pp = pre_ps.tile([128, 512], f32, tag="pp")
            nc.tensor.matmul(pp[:], lhsT=Z[:, 128 * s:128 * (s + 1)],
                             rhs=RZ[:], start=True, stop=False)
            nc.tensor.matmul(pp[:], lhsT=ZP[:, 128 * s:128 * (s + 1)],
                             rhs=RZP[:], start=False, stop=True)
            # (GPSIMD cannot access PSUM; V is idle at step start anyway)
            nc.vector.tensor_tensor(logz[:, 512 * s:512 * (s + 1)], pp[:],
                                    MBQB[:], ALU.add)

        with nc.named_scope("precompute"):
            for s in range(min(4, NSL)):
                logz_slice(s)

        # ---- recurrence ----
        for t in range(T):
            tg, tl = t // 4, t % 4
            with nc.named_scope(f"step{t}" if t % 10 == 0 else "step"):
                if 4 + t < NSL:              # stream remaining logz slices
                    logz_slice(4 + t)        # (ready long before step 4*(4+t))
                pa = attn_ps.tile([128, 512], f32, tag="pa")
                pgi = gi_ps.tile([128, 384], f32, tag="pgi")
                pgo = go_ps.tile([128, 128], f32, tag="pgo")

                # gate bias into psum (constants only; runs during prev tail)
                nc.tensor.matmul(pgi[:], lhsT=BIAS16[:], rhs=IND[:, 0:384],
                                 start=True, stop=False)
                nc.tensor.matmul(pgo[:], lhsT=BIAS16[:], rhs=IND[:, 384:512],
                                 start=True, stop=False)
                # logits: one sequential accumulation group per 32-col slice
                for j in range(4):
                    nc.tensor.matmul(pa[:, 32 * j:32 * (j + 1)],
                                     lhsT=logz[:, 512 * tg + 128 * j:512 * tg + 128 * (j + 1)],
                                     rhs=I32S[:, 32 * tl:32 * tl + 32],
                                     start=True, stop=False)
                    for k in range(4):
                        nc.tensor.matmul(pa[:, 32 * j:32 * (j + 1)],
                                         lhsT=MW[:, 128 * (4 * j + k):128 * (4 * j + k + 1)],
                                         rhs=HT[:, 32 * k:32 * k + 32],
                                         start=False, stop=(k == 3))
                # e^T = exp(logits^T)  [128 m-part, (chunk, b)]
                ET = stp.tile([128, 128], bf16, tag="ET")
                nc.scalar.activation(ET[:], pa[:, 0:128], AF.Exp)

                def whh_block(glo, ghi):
                    for g in range(glo, ghi):
                        dst = pgi if g < 12 else pgo
                        off = 32 * g if g < 12 else 32 * (g - 12)
                        for k in range(4):
                            nc.tensor.matmul(dst[:, off:off + 32],
                                             lhsT=WHH[:, 128 * (4 * g + k):128 * (4 * g + k + 1)],
                                             rhs=HT[:, 32 * k:32 * k + 32],
                                             start=False, stop=False)

                whh_block(0, 1)       # 4 MMs to cover exp latency
                # sums bcast over partitions: pa[:,128:160] = sum_m e
                for c in range(4):
                    nc.tensor.matmul(pa[:, 128:160], lhsT=ONESB[:],
                                     rhs=ET[:, 32 * c:32 * c + 32],
                                     start=(c == 0), stop=(c == 3))
                # gd^T = memory^T-chunks @ e^T
                for j in range(4):
                    nc.tensor.matmul(pa[:, 160:192],
                                     lhsT=MEMBF[:, 128 * j:128 * (j + 1)],
                                     rhs=ET[:, 32 * j:32 * j + 32],
                                     start=(j == 0), stop=(j == 3))
                recipB = stp.tile([128, 32], bf16, tag="recipB")
                with nc.allow_low_precision(reason="softmax recip in bf16 is fine"):
                    nc.vector.reciprocal(recipB[:], pa[:, 128:160])
                GDN = stp.tile([128, 32], f16, tag="GDN")
                nc.vector.tensor_tensor(GDN[:], pa[:, 160:192], recipB[:], ALU.mult)

                whh_block(1, 12)      # 44 MMs while recip/gdn run
                # W_ih @ gdn for i, f, g chunks
                for g in range(12):
                    nc.tensor.matmul(pgi[:, 32 * g:32 * (g + 1)],
                                     lhsT=WIH[:, 128 * g:128 * (g + 1)],
                                     rhs=GDN[:], start=False, stop=(g == 11))
                whh_block(12, 16)     # o chunks while tanh/c-chain run
                for g in range(12, 16):
                    nc.tensor.matmul(pgo[:, 32 * (g - 12):32 * (g - 11)],
                                     lhsT=WIH[:, 128 * g:128 * (g + 1)],
                                     rhs=GDN[:], start=False, stop=(g == 15))

                # pointwise: Y = tanh(gates_scaled), fp16 (split i,f / g for chase)
                Yif = stp.tile([128, 256], f16, tag="Yif")
                nc.scalar.activation(Yif[:], pgi[:, 0:256], AF.Tanh)
                Yg = stp.tile([128, 128], f16, tag="Yg")
                nc.scalar.activation(Yg[:], pgi[:, 256:384], AF.Tanh)
                Yo = stp.tile([128, 128], f16, tag="Yo")
                nc.scalar.activation(Yo[:], pgo[:], AF.Tanh)
                m1 = stp.tile([128, 128], f16, tag="m1")
                nc.vector.scalar_tensor_tensor(m1[:], Yif[:, 128:256], 1.0, Dt[:],
                                               ALU.add, ALU.mult)
                m2 = stp.tile([128, 128], f16, tag="m2")
                nc.vector.scalar_tensor_tensor(m2[:], Yif[:, 0:128], 1.0,
                                               Yg[:], ALU.add, ALU.mult)
                Dn = state.tile([128, 128], f16, tag="Dn")
                nc.vector.scalar_tensor_tensor(Dn[:], m1[:], 0.5, m2[:],
                                               ALU.mult, ALU.add)
                TC = stp.tile([128, 128], f16, tag="TC")
                nc.scalar.activation(TC[:], Dn[:], AF.Tanh, scale=0.5)
                Hn = state.tile([128, 128], f16, tag="Hn")
                nc.vector.scalar_tensor_tensor(Hn[:], Yo[:], 1.0, TC[:],
                                               ALU.add, ALU.mult)
                HT, Dt = Hn, Dn

        # ---- final output: out^T = 0.5*W_fc @ H + b_fc ----
        with nc.named_scope("final"):
            pf = attn_ps.tile([128, 512], f32, tag="pa")
            nc.tensor.matmul(pf[:, 0:32], lhsT=BFC1[:], rhs=ONES32[:],
                             start=True, stop=False)
            for k in range(4):
                nc.tensor.matmul(pf[:, 0:32], lhsT=WFCT[:, 128 * k:128 * (k + 1)],
                                 rhs=HT[:, 32 * k:32 * k + 32],
                                 start=False, stop=(k == 3))
            outt = stp.tile([O, BB], f32, tag="outt")
            nc.scalar.activation(outt[:], pf[:, 0:32], AF.Identity)
            nc.sync.dma_start(o_d[:], outt[:])

    nc.compile()
    return nc


def _prep_host(inputs):
    """Host-side: fold weights, compute z/zp gating streams, build in_maps."""
    f16 = np.float16
    import ml_dtypes
    b16 = ml_dtypes.bfloat16
    inp = {k: np.asarray(v, np.float32) for k, v in inputs.items()}
    mem = inp["memory"]
    Wq = inp["W_q"]
    Wq1, Wq2, Wq3 = Wq[:, :F], Wq[:, F:2 * F], Wq[:, 2 * F:]

    # gate row scaling: 0.5 for i,f,o (tanh trick), 1.0 for g; W_hh also *0.5 (H=2h)
    r = np.full((4 * H, 1), 0.5, np.float32)
    r[2 * H:3 * H] = 1.0
    WIHs = r * inp["W_ih"]
    WHHs = r * inp["W_hh"] * 0.5
    biass = r[:, 0] * (inp["b_ih"] + inp["b_hh"])

    wih = np.empty((128, 16 * 128), np.float32)
    for g in range(16):
        wih[:, 128 * g:128 * (g + 1)] = WIHs[128 * g:128 * (g + 1), :].T
    whh = np.empty((128, 64 * 128), np.float32)
    for g in range(16):
        for k in range(4):
            whh[:, 128 * (4 * g + k):128 * (4 * g + k + 1)] = \
                WHHs[128 * g:128 * (g + 1), 128 * k:128 * (k + 1)].T
    MWmat = 0.5 * (mem @ (Wq3 @ inp["W_fc"]))          # [M, H]
    mw = np.empty((128, 16 * 128), np.float32)
    for j in range(4):
        for k in range(4):
            mw[:, 128 * (4 * j + k):128 * (4 * j + k + 1)] = \
                MWmat[128 * j:128 * (j + 1), 128 * k:128 * (k + 1)].T
    membf = np.empty((128, 512), np.float32)
    for j in range(4):
        membf[:, 128 * j:128 * (j + 1)] = mem[128 * j:128 * (j + 1), :]
    wfct = np.empty((128, 512), np.float32)
    WFCs = (0.5 * inp["W_fc"]).T                       # [H, O]
    for k in range(4):
        wfct[:, 128 * k:128 * (k + 1)] = WFCs[128 * k:128 * (k + 1), :]

    bias16 = biass.reshape(16, 128)
    ind = np.zeros((16, 512), np.float32)
    for g in range(16):
        ind[g, 32 * g:32 * (g + 1)] = 1.0
    i32sel = np.eye(128, dtype=f16)
    mbq = mem @ (inp["b_q"] + Wq3 @ inp["b_fc"])       # [M]
    mbqb = np.broadcast_to(mbq[None, :], (128, 512)).copy()
    bfc1 = inp["b_fc"][None, :]

    shared = dict(
        mw=mw.astype(f16), whh=whh.astype(f16), wih=wih.astype(f16),
        membf=membf.astype(b16), rz=(mem @ Wq1).T.astype(f16),
        rzp=(mem @ Wq2).T.astype(f16), wfct=wfct.astype(f16),
        bias16=bias16.astype(f16), ind=ind.astype(f16), i32sel=i32sel,
        mbqb=mbqb.astype(f16), bfc1=bfc1.astype(f16),
    )

    # z/zp gating streams on host (input-only elementwise preprocessing)
    x = inp["input"]                                   # [B, 6, 100, F]
    X, Xl, Mask, Delta, Xlb, Dltb = (x[:, i, :T] for i in range(6))
    Xm = inp["X_mean"][None, :T, :]                    # [1, T, F]
    dgz = np.diag(inp["W_gz"])[None, None, :]
    bgz = inp["b_gz"][None, None, :]
    dgzp = np.diag(inp["W_gzp"])[None, None, :]
    bgzp = inp["b_gzp"][None, None, :]
    dz = np.minimum(np.exp(-dgz * Delta - bgz), 1.0)
    dzp = np.minimum(np.exp(-dgzp * Dltb - bgzp), 1.0)
    zfull = Mask * X + (1 - Mask) * (dz * Xl + (1 - dz) * Xm)      # [B, T, F]
    zpfull = Mask * X + (1 - Mask) * (dzp * Xlb + (1 - dzp) * Xm)

    in_maps = []
    for core in range(NC):
        b0 = core * BB
        m_ = dict(shared)
        # [BB, T, F] -> [F, T*BB]
        m_["z"] = np.ascontiguousarray(
            np.transpose(zfull[b0:b0 + BB], (2, 1, 0)).reshape(F, TB)).astype(f16)
        m_["zp"] = np.ascontiguousarray(
            np.transpose(zpfull[b0:b0 + BB], (2, 1, 0)).reshape(F, TB)).astype(f16)
        in_maps.append(m_)
    return in_maps


def kernel(**inputs):
    global _built
    from concourse import bass_utils
    if _built is None:
        _built = _build()
    in_maps = _prep_host(inputs)
    res = bass_utils.run_bass_kernel_spmd(_built, in_maps, core_ids=list(range(NC)))
    out = np.empty((B, 1, O), np.float32)
    for core in range(NC):
        out[core * BB:(core + 1) * BB, 0, :] = res.results[core]["o"].T
    return out
